# revision 13
# baseline (speedup 1.0000x reference)
"""Trainium2 Bass kernel for nn_CrossAttention (B=2, T=2048, D=1024, H=16, hd=64).

Sharding: 32 (batch, head) units over 8 cores -> each core handles 1 batch and
4 contiguous heads (core c: batch c//4, heads (c%4)*4 .. +4).  Per-core kernel
computes the partial c_proj output for its 4 heads; host sums the 4 partials
per batch and adds bc.

Per-core dataflow (bf16 operands, fp32 PSUM accumulation; activations
transposed, D-on-partitions):
  qpT/kpT [256, 2048] = W.T @ xT (+bias)   (heads 2j,2j+1 stacked in group j)
  attention pass = (pair j, tq-quarter q of 512):
    per tv-chunk mv (128):
      S(h0) rows 0-63 / S(h1) rows 64-127 issued back-to-back -> concurrent
        PE row groups; both land in one [128,1024] PSUM tile (h0|h1)
      exp via one ScalarE activation [128,1024] (scale 1/8 fused) -> the
        ScalarE stream is the kernel's critical resource (~143us)
      y_ext[h] += [v_h | ones64].T @ es_h  -> rows 0-63 y, rows 64-127 the
        softmax denominator replicated (free partition-broadcast)
    normalize: one DVE copy frees the psum slot fast, then reciprocal +
      tensor_tensor mult into yallT off the critical path
  c_proj accumulated over j in PSUM (single out tensor).

Scheduling: the cp1 projections and the c_proj row-chunks are injected as
small "filler" units into the attention mv loops so the PE stream stays dense
and ScalarE never starves (PE queue is FIFO per engine).
"""

import sys

sys.path.insert(0, "/opt/trn_rl_repo")

import ml_dtypes
import numpy as np

bf = ml_dtypes.bfloat16

import concourse.bacc as bacc
import concourse.bass as bass
import concourse.mybir as mybir
import concourse.tile as tile
from concourse.bass_utils import run_bass_kernel_spmd

F32 = mybir.dt.float32
BF16 = mybir.dt.bfloat16

T = 2048          # sequence length (both q and kv)
D = 1024          # model dim
HL = 4            # heads per core
HD = 64           # head dim
DH = HL * HD      # 256 local projected dim
P = 128
MV = T // P       # 16 tv chunks
SCALE = 1.0 / 8.0  # 1/sqrt(64)

N_CORES = 8

_cache = {}


def build_nc():
    if "nc" in _cache:
        return _cache["nc"]
    nc = bacc.Bacc(
        "TRN2",
        target_bir_lowering=False,
        debug=False,
        num_devices=N_CORES,
    )

    qT = nc.declare_dram_parameter("qT", [D, T], BF16, isOutput=False)
    kT = nc.declare_dram_parameter("kT", [D, T], BF16, isOutput=False)
    v_sl = nc.declare_dram_parameter("v_sl", [P, T // P, DH], BF16, isOutput=False)
    WqT = nc.declare_dram_parameter("WqT", [P, D // P, DH], BF16, isOutput=False)
    WkT = nc.declare_dram_parameter("WkT", [P, D // P, DH], BF16, isOutput=False)
    WcT = nc.declare_dram_parameter("WcT", [P, DH // P, D], BF16, isOutput=False)
    bqk = nc.declare_dram_parameter("bqk", [P, 4], F32, isOutput=False)
    out = nc.declare_dram_parameter("out", [T, D], F32, isOutput=True)

    KT = D // P   # 8 din tiles
    JT = DH // P  # 2 dout tiles (head pairs)

    with tile.TileContext(nc) as tc:
        with (
            tc.tile_pool(name="wpool", bufs=1) as wpool,
            tc.tile_pool(name="stream", bufs=16) as stream,
            tc.tile_pool(name="projsb", bufs=1) as projsb,
            tc.tile_pool(name="vpool", bufs=1) as vpool,
            tc.tile_pool(name="epool", bufs=4) as epool,
            tc.tile_pool(name="rpool", bufs=6) as rpool,
            tc.tile_pool(name="opool", bufs=3) as opool,
            tc.tile_pool(name="psA", bufs=2, space="PSUM") as psA,
            tc.tile_pool(name="psB", bufs=4, space="PSUM") as psB,
        ):
            # ---- weights / constants (k-proj inputs first: PE starts ~5us) ----
            wk_sb = wpool.tile([P, KT, DH], BF16, name="wk_sb")
            nc.sync.dma_start(wk_sb[:], WkT.ap())
            bias_sb = wpool.tile([P, 4], F32, name="bias_sb")  # [bq0,bq1,bk0,bk1]
            nc.sync.dma_start(bias_sb[:], bqk.ap())

            # ---- whole-pass projection (used for the upfront cp0 passes) ----
            def project_cp(xT_dram, w_sb, bias_col0, name, xpT, cp):
                xt_tiles = []
                for i in range(KT):
                    xt = stream.tile([P, 1024], BF16, tag="xt",
                                     name=f"{name}t{cp}_{i}")
                    nc.sync.dma_start(
                        xt[:],
                        xT_dram.ap()[i * P:(i + 1) * P,
                                     cp * 1024:(cp + 1) * 1024],
                    )
                    xt_tiles.append(xt)
                groups = [
                    psA.tile([P, 1024], F32, tag="psA", name=f"{name}p{j}{cp}")
                    for j in range(JT)
                ]
                for i in range(KT):
                    for j in range(JT):
                        for c in range(2):
                            nc.tensor.matmul(
                                groups[j][:, c * 512:(c + 1) * 512],
                                w_sb[:, i, j * P:(j + 1) * P],
                                xt_tiles[i][:, c * 512:(c + 1) * 512],
                                start=(i == 0),
                                stop=(i == KT - 1),
                            )
                for j in range(JT):
                    nc.vector.tensor_tensor(
                        xpT[:, j, cp * 1024:(cp + 1) * 1024],
                        groups[j][:],
                        bias_sb[:, bias_col0 + j:bias_col0 + j + 1]
                        .to_broadcast((P, 1024)),
                        mybir.AluOpType.add,
                    )

            # ---- unit-split projection for one (cp, j): filler-injectable.
            # Holds one psA slot only; xt tiles shared between the j chains.
            def proj_units(xT_dram, w_sb, bias_col0, name, xpT, cp, j, state):
                def u_start():
                    if "x" not in state:
                        xts = []
                        for i in range(KT):
                            xt = stream.tile([P, 1024], BF16, tag="xt",
                                             name=f"{name}u{cp}_{i}")
                            nc.sync.dma_start(
                                xt[:],
                                xT_dram.ap()[i * P:(i + 1) * P,
                                             cp * 1024:(cp + 1) * 1024],
                            )
                            xts.append(xt)
                        state["x"] = xts
                    state[j] = psA.tile([P, 1024], F32, tag="psA",
                                        name=f"{name}g{cp}{j}")
                    u_mm(0)

                def u_mm(i):
                    g = state[j]
                    for c in range(2):
                        nc.tensor.matmul(
                            g[:, c * 512:(c + 1) * 512],
                            w_sb[:, i, j * P:(j + 1) * P],
                            state["x"][i][:, c * 512:(c + 1) * 512],
                            start=(i == 0),
                            stop=(i == KT - 1),
                        )

                def u_evac():
                    nc.vector.tensor_tensor(
                        xpT[:, j, cp * 1024:(cp + 1) * 1024],
                        state[j][:],
                        bias_sb[:, bias_col0 + j:bias_col0 + j + 1]
                        .to_broadcast((P, 1024)),
                        mybir.AluOpType.add,
                    )

                units = [(0, u_start)]
                for i in range(1, KT):
                    units.append((0, (lambda i=i: u_mm(i))))
                units.append((0, u_evac))
                return units

            kpT = projsb.tile([P, JT, T], BF16, name="kpT")
            qpT = projsb.tile([P, JT, T], BF16, name="qpT")
            project_cp(kT, wk_sb, 2, "k", kpT, 0)
            wq_sb = wpool.tile([P, KT, DH], BF16, name="wq_sb")
            nc.sync.dma_start(wq_sb[:], WqT.ap())

            v_all = vpool.tile([P, MV, DH], BF16, name="v_all")
            nc.sync.dma_start(v_all[:], v_sl.ap())

            project_cp(qT, wq_sb, 0, "q", qpT, 0)

            vext = []
            for h in range(HL):
                # [v_h | ones64]: ones block makes the y matmul emit the
                # softmax denominator replicated on psum rows 64-127
                ve = vpool.tile([P, MV, P], BF16, name=f"vext{h}")
                nc.vector.tensor_copy(ve[:, :, 0:HD],
                                      v_all[:, :, h * HD:(h + 1) * HD])
                nc.vector.memset(ve[:, :, HD:P], 1.0)
                vext.append(ve)

            project_cp(kT, wk_sb, 2, "k", kpT, 1)
            project_cp(qT, wq_sb, 0, "q", qpT, 1)
            wc_sb = wpool.tile([P, JT, D], BF16, name="wc_sb")
            nc.sync.dma_start(wc_sb[:], WcT.ap())

            yallT = projsb.tile([P, JT, T], BF16, name="yallT")

            # ---- attention pass: head pair j, tq quarter q (512 cols).
            # Supports split mv ranges (shared yb accumulation) and filler
            # units (min_mv, fn) injected one per mv before the S matmuls.
            def attn_pass(j, q, fillers=(), mv_lo=0, mv_hi=MV, yb=None,
                          chunked_norm=False):
                fillers = list(fillers)
                qoff = q * 512
                if yb is None:
                    yb = [
                        psB.tile([P, 512], F32, tag="psB", name=f"y{2*j+hi}_{q}")
                        for hi in range(2)
                    ]
                for mv in range(mv_lo, mv_hi):
                    if fillers and fillers[0][0] <= mv:
                        fillers.pop(0)[1]()
                    s = psA.tile([P, 1024], F32, tag="psA",
                                 name=f"s{j}_{q}_{mv}")
                    # two heads' S chunks back-to-back -> concurrent row
                    # groups (0,0) and (64,0) on the PE array
                    nc.tensor.matmul(
                        s[:, 0:512],
                        kpT[0:HD, j, mv * P:(mv + 1) * P],
                        qpT[0:HD, j, qoff:qoff + 512],
                        start=True, stop=True,
                    )
                    nc.tensor.matmul(
                        s[:, 512:1024],
                        kpT[HD:P, j, mv * P:(mv + 1) * P],
                        qpT[HD:P, j, qoff:qoff + 512],
                        start=True, stop=True,
                    )
                    es = epool.tile([P, 1024], BF16, tag="es",
                                    name=f"e{j}_{q}_{mv}")
                    nc.scalar.activation(
                        es[:], s[:], mybir.ActivationFunctionType.Exp,
                        scale=SCALE,
                    )
                    for hi in range(2):
                        nc.tensor.matmul(
                            yb[hi][:],
                            vext[2 * j + hi][:, mv, :],
                            es[:, hi * 512:(hi + 1) * 512],
                            start=(mv == 0),
                            stop=(mv == MV - 1),
                        )
                for _, f in fillers:
                    f()
                if mv_hi < MV:
                    return yb
                # normalize: both fast copies first (frees both psum
                # slots in ~1.4us), then recip + mult off the critical path
                ycs = []
                for hi in range(2):
                    yc = rpool.tile([P, 512], F32, tag="yc",
                                    name=f"yc{j}_{q}_{hi}")
                    nc.vector.tensor_copy(yc[:], yb[hi][:])
                    ycs.append(yc)
                chunks = [(0, 512)] if not chunked_norm else [
                    (c * 128, 128) for c in range(4)
                ]
                for c0, cw in chunks:
                    for hi in range(2):
                        rc = rpool.tile([HD, 512], F32, tag="rc",
                                        name=f"rc{j}_{q}_{hi}_{c0}")
                        nc.vector.reciprocal(rc[:, 0:cw],
                                             ycs[hi][HD:P, c0:c0 + cw])
                        nc.vector.tensor_tensor(
                            yallT[hi * HD:(hi + 1) * HD, j,
                                  qoff + c0:qoff + c0 + cw],
                            ycs[hi][0:HD, c0:c0 + cw],
                            rc[:, 0:cw],
                            mybir.AluOpType.mult,
                        )
                return yb

            # ---- c_proj unit: one tq row chunk mt, j-accumulated ----
            def cproj_unit(mt):
                def emit():
                    o_sb = opool.tile([P, 1024], F32, tag="osb",
                                      name=f"ot{mt}")
                    for nch in range(2):
                        o_ps = psB.tile([P, 512], F32, tag="psB",
                                        name=f"o{mt}_{nch}")
                        for j in range(JT):
                            nc.tensor.matmul(
                                o_ps[:],
                                yallT[:, j, mt * P:(mt + 1) * P],
                                wc_sb[:, j, nch * 512:(nch + 1) * 512],
                                start=(j == 0),
                                stop=(j == JT - 1),
                            )
                        nc.vector.tensor_copy(
                            o_sb[:, nch * 512:(nch + 1) * 512], o_ps[:]
                        )
                    nc.sync.dma_start(out.ap()[mt * P:(mt + 1) * P, :], o_sb[:])
                    # two dummy allocations keep the pool round-robin aligned
                    # so real o_ps tiles land on the PREVIOUS pass's freed yb
                    # slots, never on the current pass's live ones
                    psB.tile([P, 512], F32, tag="psB", name=f"dum{mt}a")
                    psB.tile([P, 512], F32, tag="psB", name=f"dum{mt}b")
                return emit

            def cproj_units(q, min_mv=6):
                return [(min_mv + 2 * i, cproj_unit(mt))
                        for i, mt in enumerate(range(4 * q, 4 * q + 4))]

            attn_pass(0, 0)
            attn_pass(1, 0)
            attn_pass(0, 1)
            attn_pass(1, 1, cproj_units(0))
            attn_pass(0, 2)
            attn_pass(1, 2, cproj_units(1))
            attn_pass(0, 3)
            attn_pass(1, 3, cproj_units(2), chunked_norm=True)
            for _, u in cproj_units(3):
                u()

    nc.compile()
    _cache["nc"] = nc
    return nc


def make_in_maps(k, q, v, Wk, bk, Wq, bq, Wc, bc):
    k = np.asarray(k, dtype=np.float32)
    q = np.asarray(q, dtype=np.float32)
    v = np.asarray(v, dtype=np.float32)
    Wk = np.asarray(Wk, dtype=np.float32)
    Wq = np.asarray(Wq, dtype=np.float32)
    Wc = np.asarray(Wc, dtype=np.float32)
    bk = np.asarray(bk, dtype=np.float32)
    bq = np.asarray(bq, dtype=np.float32)
    in_maps = []
    for c in range(N_CORES):
        b = c // 4
        h0 = (c % 4) * HL
        sl = slice(h0 * HD, h0 * HD + DH)
        bq_t = np.ascontiguousarray(bq[sl].reshape(2, P).T)  # [128, 2]
        bk_t = np.ascontiguousarray(bk[sl].reshape(2, P).T)
        bqk = np.concatenate([bq_t, bk_t], axis=1)           # [128, 4]
        def pam(w):  # [(a p), m] -> [p, a, m]
            return np.ascontiguousarray(
                w.reshape(-1, P, w.shape[1]).transpose(1, 0, 2)).astype(bf)

        in_maps.append({
            "qT": np.ascontiguousarray(q[b].T).astype(bf),
            "kT": np.ascontiguousarray(k[b].T).astype(bf),
            "v_sl": pam(v[b][:, sl]),              # [(t p), d] -> [p, t, d]
            "WqT": pam(Wq[sl, :].T),
            "WkT": pam(Wk[sl, :].T),
            "WcT": pam(Wc[:, sl].T),
            "bqk": np.ascontiguousarray(bqk),
        })
    return in_maps


def kernel(k, q, v, Wk, bk, Wq, bq, Wc, bc, _trace=False, _trace_cores=None):
    bc = np.asarray(bc, dtype=np.float32)
    nc = build_nc()
    in_maps = make_in_maps(k, q, v, Wk, bk, Wq, bq, Wc, bc)
    res = run_bass_kernel_spmd(
        nc, in_maps, core_ids=list(range(N_CORES)),
        trace=_trace, trace_cores=_trace_cores,
    )
    outs = [res.results[c]["out"] for c in range(N_CORES)]
    full = np.stack([
        outs[0] + outs[1] + outs[2] + outs[3],
        outs[4] + outs[5] + outs[6] + outs[7],
    ]) + bc[None, None, :]
    kernel.last_result = res
    return full.astype(np.float32)
